# revision 8
# baseline (speedup 1.0000x reference)
"""Disentangled spatial attention TRN2 kernel (8 NeuronCores).

Sharding: 8 cores = 2 batches x 4 head-groups (4 heads each).
Per core, transposed-activation layout:
  qcat[h] (128, L):  rows 0:64 qt_h, rows 64:128 qs_h
  kcat[h] (128, L):  rows 0:64 k1_h = kt + lam_ts*ks,
                     rows 64:128 k2_h = lam_st*kt + lam_ss*ks
  scores^T chunk = kcat_chunk.T @ qcat  (both reference score einsums
  fused into one K=128 matmul; lam_* folded into weight shards on host)
  softmax row-sums ride along the PV matmul as 64 replicated "ones"
  columns of the v operand; normalization happens on the way into the
  transposed y layout that feeds the output projection.
Phase-1 matmuls run in float32r; attention + projection operands are
bf16 (fp32 PSUM accumulation).  v/c biases are folded in on the host
(exact: softmax rows sum to 1), qkv biases are added on device.
"""
import numpy as np
import ml_dtypes
import concourse.bass as bass
import concourse.mybir as mybir
import concourse.tile as tile
from concourse.bass_utils import run_bass_kernel_spmd

F32 = mybir.dt.float32
F32R = mybir.dt.float32r
BF16 = mybir.dt.bfloat16
AF = mybir.ActivationFunctionType

B, L, E, H, D = 2, 2048, 1024, 16, 64
HPC = 4          # heads per core
NCORES = 8
LTB = 512        # L block for phase 1
NLTB = L // LTB  # 4
NCHUNK = L // 128  # 16 Lk chunks
EC = E // 128    # 8 E chunks


def _split_multi_waits(nc, max_waits=1):
    """walrus codegen allows only one sync wait per instruction; move extra
    waits onto standalone same-engine NoOps placed just before."""
    n_split = 0
    for f in nc.m.functions:
        for blk in f.blocks:
            insts = list(blk.instructions)
            out = []
            changed = False
            for inst in insts:
                si = inst.sync_info
                waits = list(si.on_wait) if si is not None and si.on_wait else []
                if len(waits) > max_waits:
                    keep = waits[-max_waits:]
                    extra = waits[:-max_waits]
                    for w in extra:
                        nop = mybir.InstNoOp(
                            name=f"{inst.name}-wsplit{n_split}",
                            engine=inst.engine,
                            ins=[], outs=[],
                            sync_info=mybir.SyncInfo(on_wait=[w], on_update=[]),
                        )
                        out.append(nop)
                        n_split += 1
                    inst.sync_info = mybir.SyncInfo(
                        on_wait=keep,
                        on_update=list(si.on_update) if si.on_update else [],
                    )
                    changed = True
                out.append(inst)
            if changed:
                blk.instructions = out
    return n_split


def _build():
    nc = bass.Bass()
    xtT = nc.declare_dram_parameter("xtT", [E, L], F32R, isOutput=False)
    xsT = nc.declare_dram_parameter("xsT", [E, L], F32R, isOutput=False)
    wq = nc.declare_dram_parameter("wq", [128, EC, HPC * D], F32R, isOutput=False)
    wqs = nc.declare_dram_parameter("wqs", [128, EC, HPC * D], F32R, isOutput=False)
    wk = nc.declare_dram_parameter("wk", [128, 2 * EC, 2 * HPC * D], F32R, isOutput=False)
    wv = nc.declare_dram_parameter("wv", [128, EC, HPC * D], F32R, isOutput=False)
    wc = nc.declare_dram_parameter("wc", [128, 2, E], BF16, isOutput=False)
    bq = nc.declare_dram_parameter("bq", [128, 2], F32, isOutput=False)
    bqs = nc.declare_dram_parameter("bqs", [128, 2], F32, isOutput=False)
    bk = nc.declare_dram_parameter("bk", [128, HPC], F32, isOutput=False)
    ones = nc.declare_dram_parameter("ones", [128, NCHUNK, 2, 64], BF16,
                                     isOutput=False)
    out = nc.declare_dram_parameter("out", [L, E], F32, isOutput=True)

    xtT_v = xtT.rearrange("(k p) l -> p k l", p=128)   # (128, 8, L)
    xsT_v = xsT.rearrange("(k p) l -> p k l", p=128)

    with tile.TileContext(nc) as tc:
        with tc.tile_pool(name="wpool", bufs=1) as wpool, \
             tc.tile_pool(name="persist", bufs=1) as pp:
            # persistent activations (bf16)
            qcat = [pp.tile([128, L], BF16, tag=f"qcat{h}", name=f"qcat{h}")
                    for h in range(HPC)]
            kcat = [pp.tile([128, L], BF16, tag=f"kcat{h}", name=f"kcat{h}")
                    for h in range(HPC)]
            # v_aug: (128, chunk, head, 128); head slot s=0: [ones | v],
            # s=1: [v | ones]
            v_sb = pp.tile([128, NCHUNK, HPC, 128], BF16, name="v_sb")
            yT = [pp.tile([128, L], BF16, tag=f"yT{j}", name=f"yT{j}")
                  for j in range(2)]
            # staging for partition-shifted qcat halves: per pair j,
            # rows 64:128 <- qt_{2j+1}, rows 0:64 <- qs_{2j}
            qstg = [pp.tile([128, L], BF16, tag=f"qstg{j}", name=f"qstg{j}")
                    for j in range(2)]

            wq_sb = wpool.tile([128, EC, HPC * D], F32R)
            wqs_sb = wpool.tile([128, EC, HPC * D], F32R)
            wk_sb = wpool.tile([128, 2 * EC, 2 * HPC * D], F32R)
            wv_sb = wpool.tile([128, EC, HPC * D], F32R)
            bq_sb = wpool.tile([128, 2], F32)
            bqs_sb = wpool.tile([128, 2], F32)
            bk_sb = wpool.tile([128, HPC], F32)

            # ---------------- phase 1: QKV ----------------
            with tc.tile_pool(name="xp", bufs=2) as xp, \
                 tc.tile_pool(name="p1ps", bufs=2, space="PSUM") as p1q, \
                 tc.tile_pool(name="p1psk", bufs=1, space="PSUM") as p1k, \
                 tc.tile_pool(name="p1psv", bufs=2, space="PSUM") as p1v:
                first = True
                for ltb in range(NLTB):
                    ls = slice(ltb * LTB, (ltb + 1) * LTB)
                    xt_blk = xp.tile([128, EC, LTB], F32R, tag="x",
                                     name=f"xt{ltb}")
                    nc.sync.dma_start(xt_blk[:], xtT_v[:, :, ls])
                    if first:
                        # x block first, then weights in use order
                        nc.sync.dma_start(wq_sb[:], wq[:])
                        nc.sync.dma_start(bq_sb[:], bq[:])
                        nc.sync.dma_start(wk_sb[:], wk[:])
                        nc.sync.dma_start(wv_sb[:], wv[:])
                        nc.sync.dma_start(wqs_sb[:], wqs[:])
                        nc.sync.dma_start(bqs_sb[:], bqs[:])
                        nc.sync.dma_start(bk_sb[:], bk[:])
                        nc.sync.dma_start(v_sb[:, :, 0::2, 0:64], ones[:])
                        nc.sync.dma_start(v_sb[:, :, 1::2, 64:128], ones[:])
                        first = False

                    # qt for head pairs
                    for j in range(2):
                        pq = p1q.tile([128, LTB], F32, tag="pq", name=f"pq{ltb}{j}")
                        for k in range(EC):
                            nc.tensor.matmul(
                                pq[:], wq_sb[:, k, j * 128:(j + 1) * 128],
                                xt_blk[:, k, :],
                                start=(k == 0), stop=(k == EC - 1),
                                skip_group_check=True)
                        nc.vector.tensor_scalar_add(
                            qcat[2 * j][0:64, ls], pq[0:64, :], bq_sb[0:64, j:j + 1])
                        nc.vector.tensor_scalar_add(
                            qstg[j][64:128, ls], pq[64:128, :], bq_sb[64:128, j:j + 1])

                    # k stage A (xt part) — psums stay open
                    pk = []
                    for h in range(HPC):
                        pkh = p1k.tile([128, LTB], F32, tag=f"pk{h}",
                                       name=f"pk{ltb}{h}")
                        pk.append(pkh)
                        for k in range(EC):
                            nc.tensor.matmul(
                                pkh[:], wk_sb[:, k, h * 128:(h + 1) * 128],
                                xt_blk[:, k, :],
                                start=(k == 0), stop=False,
                                skip_group_check=True)

                    # v (natural layout) into v_aug slots
                    for vt in range(LTB // 128):
                        ck = ltb * (LTB // 128) + vt
                        pv = p1v.tile([128, HPC * D], F32, tag="pv",
                                      name=f"pv{ck}")
                        for k in range(EC):
                            nc.tensor.matmul(
                                pv[:], xt_blk[:, k, vt * 128:(vt + 1) * 128],
                                wv_sb[:, k, :],
                                start=(k == 0), stop=(k == EC - 1),
                                skip_group_check=True)
                        pv_v = pv.rearrange("p (h d) -> p h d", d=D)
                        # even heads (slot 0): v in cols 64:128
                        nc.vector.tensor_copy(v_sb[:, ck, 0::2, 64:128],
                                              pv_v[:, 0::2, :])
                        # odd heads (slot 1): v in cols 0:64
                        nc.vector.tensor_copy(v_sb[:, ck, 1::2, 0:64],
                                              pv_v[:, 1::2, :])

                    xs_blk = xp.tile([128, EC, LTB], F32R, tag="x",
                                     name=f"xs{ltb}")
                    nc.sync.dma_start(xs_blk[:], xsT_v[:, :, ls])

                    # qs for head pairs
                    for j in range(2):
                        pq = p1q.tile([128, LTB], F32, tag="pq",
                                      name=f"pqs{ltb}{j}")
                        for k in range(EC):
                            nc.tensor.matmul(
                                pq[:], wqs_sb[:, k, j * 128:(j + 1) * 128],
                                xs_blk[:, k, :],
                                start=(k == 0), stop=(k == EC - 1),
                                skip_group_check=True)
                        nc.vector.tensor_scalar_add(
                            qstg[j][0:64, ls], pq[0:64, :], bqs_sb[0:64, j:j + 1])
                        nc.vector.tensor_scalar_add(
                            qcat[2 * j + 1][64:128, ls], pq[64:128, :],
                            bqs_sb[64:128, j:j + 1])

                    # k stage B (xs part) + copy out
                    for h in range(HPC):
                        for k in range(EC):
                            nc.tensor.matmul(
                                pk[h][:], wk_sb[:, EC + k, h * 128:(h + 1) * 128],
                                xs_blk[:, k, :],
                                start=False, stop=(k == EC - 1),
                                skip_group_check=True)
                        nc.vector.tensor_scalar_add(
                            kcat[h][:, ls], pk[h][:], bk_sb[:, h:h + 1])

                # resolve staged qcat halves (partition shifts via DMA)
                for j in range(2):
                    nc.sync.dma_start(qcat[2 * j + 1][0:64, :], qstg[j][64:128, :])
                    nc.sync.dma_start(qcat[2 * j][64:128, :], qstg[j][0:64, :])

            # ---------------- phase 2: attention ----------------
            with tc.tile_pool(name="expp", bufs=6) as expp, \
                 tc.tile_pool(name="np2", bufs=2) as np2, \
                 tc.tile_pool(name="p2s", bufs=2, space="PSUM") as p2s, \
                 tc.tile_pool(name="p2y", bufs=2, space="PSUM") as p2y:
                for h in range(HPC):
                    j, s = h // 2, h % 2
                    sums_h = slice(0, 64) if s == 0 else slice(64, 128)
                    y_h = slice(64, 128) if s == 0 else slice(0, 64)
                    slot = slice(0, 64) if s == 0 else slice(64, 128)
                    for lq in range(2):
                        qs_ = slice(lq * 1024, (lq + 1) * 1024)
                        py = p2y.tile([128, 1024], F32, tag="py",
                                      name=f"py{h}{lq}")
                        for ck in range(NCHUNK):
                            ps = p2s.tile([128, 1024], F32, tag="ps",
                                          name=f"ps{h}{lq}{ck}")
                            for hf in range(2):
                                nc.tensor.matmul(
                                    ps[:, hf * 512:(hf + 1) * 512],
                                    kcat[h][:, ck * 128:(ck + 1) * 128],
                                    qcat[h][:, lq * 1024 + hf * 512:
                                            lq * 1024 + (hf + 1) * 512],
                                    start=True, stop=True,
                                    skip_group_check=True)
                            ex = expp.tile([128, 1024], BF16, tag="ex",
                                           name=f"ex{h}{lq}{ck}")
                            nc.scalar.activation(ex[:], ps[:], AF.Exp,
                                                 scale=0.125)
                            for hf in range(2):
                                nc.tensor.matmul(
                                    py[:, hf * 512:(hf + 1) * 512],
                                    v_sb[:, ck, h, :],
                                    ex[:, hf * 512:(hf + 1) * 512],
                                    start=(ck == 0), stop=(ck == NCHUNK - 1),
                                    skip_group_check=True)
                        # normalize: recip of sums (DVE or ACT, balanced);
                        # DMA-shift recip to y partitions; TT multiply
                        # PSUM -> staging; DMA staging into the yT slot.
                        rec = np2.tile([128, 1024], F32, tag="rec",
                                       name=f"rec{h}{lq}")
                        if h % 2 == 0:
                            nc.vector.reciprocal(rec[sums_h, :], py[sums_h, :])
                        else:
                            lnt = np2.tile([128, 1024], F32, tag="lnt",
                                           name=f"ln{h}{lq}")
                            nc.scalar.activation(lnt[sums_h, :], py[sums_h, :],
                                                 AF.Ln)
                            nc.scalar.activation(rec[sums_h, :], lnt[sums_h, :],
                                                 AF.Exp, scale=-1.0)
                        rec2 = np2.tile([128, 1024], F32, tag="rec2",
                                        name=f"rec2{h}{lq}")
                        nc.sync.dma_start(rec2[y_h, :], rec[sums_h, :])
                        yst = np2.tile([128, 1024], BF16, tag="yst",
                                       name=f"yst{h}{lq}")
                        nc.vector.tensor_tensor(yst[y_h, :], py[y_h, :],
                                                rec2[y_h, :],
                                                mybir.AluOpType.mult)
                        nc.sync.dma_start(yT[j][slot, qs_], yst[y_h, :])

            # ---------------- phase 3: projection ----------------
            with tc.tile_pool(name="wc3", bufs=1) as wc3p, \
                 tc.tile_pool(name="outp", bufs=3) as outp, \
                 tc.tile_pool(name="p3o", bufs=4, space="PSUM") as p3o:
                wc_sb = wc3p.tile([128, 2, E], BF16)
                nc.sync.dma_start(wc_sb[:], wc[:])
                for lqt in range(L // 128):
                    lqs = slice(lqt * 128, (lqt + 1) * 128)
                    ot = outp.tile([128, E], F32, tag="ot", name=f"ot{lqt}")
                    for nch in range(2):
                        ns = slice(nch * 512, (nch + 1) * 512)
                        po = p3o.tile([128, 512], F32, tag="po",
                                      name=f"po{lqt}{nch}")
                        nc.tensor.matmul(po[:], yT[0][:, lqs], wc_sb[:, 0, ns],
                                         start=True, stop=False,
                                         skip_group_check=True)
                        nc.tensor.matmul(po[:], yT[1][:, lqs], wc_sb[:, 1, ns],
                                         start=False, stop=True,
                                         skip_group_check=True)
                        if nch == 0:
                            nc.vector.tensor_copy(ot[:, ns], po[:])
                        else:
                            nc.scalar.copy(ot[:, ns], po[:])
                    nc.sync.dma_start(out[lqs, :], ot[:])
    return nc


_NC_CACHE = None


def _get_nc():
    global _NC_CACHE
    if _NC_CACHE is None:
        nc = _build()
        _split_multi_waits(nc)
        _NC_CACHE = nc
    return _NC_CACHE


def _prep_core_inputs(core, xt, xs, Wt, bt, Ws, bs, Wc, bc, lam_ts, lam_st,
                      lam_ss):
    b, hg = core // HPC, core % HPC
    c0 = hg * HPC * D  # 256*hg
    lts, lst, lss = float(lam_ts[0]), float(lam_st[0]), float(lam_ss[0])

    wq_full = Wt[:, c0:c0 + HPC * D]                     # (E, 256) qt
    wqs_full = Ws[:, c0:c0 + HPC * D]                    # (E, 256) qs
    wv_full = Wt[:, 2 * E + c0:2 * E + c0 + HPC * D]     # (E, 256)
    ktw = Wt[:, E + c0:E + c0 + HPC * D]                 # (E, 256)
    ksw = Ws[:, E + c0:E + c0 + HPC * D]                 # (E, 256)

    wk_full = np.zeros((2 * E, 2 * HPC * D), np.float32)
    for h in range(HPC):
        hs = slice(h * D, (h + 1) * D)
        wk_full[:E, h * 128:h * 128 + D] = ktw[:, hs]
        wk_full[:E, h * 128 + D:(h + 1) * 128] = lst * ktw[:, hs]
        wk_full[E:, h * 128:h * 128 + D] = lts * ksw[:, hs]
        wk_full[E:, h * 128 + D:(h + 1) * 128] = lss * ksw[:, hs]

    def chunked(a, nk, dtype=np.float32):
        return np.ascontiguousarray(
            a.reshape(nk, 128, a.shape[1]).transpose(1, 0, 2)).astype(dtype)

    btq = bt[c0:c0 + HPC * D]
    bsq = bs[c0:c0 + HPC * D]
    btk = bt[E + c0:E + c0 + HPC * D]
    bsk = bs[E + c0:E + c0 + HPC * D]
    bq_arr = np.zeros((128, 2), np.float32)
    bqs_arr = np.zeros((128, 2), np.float32)
    bk_arr = np.zeros((128, HPC), np.float32)
    for j in range(2):
        bq_arr[0:64, j] = btq[(2 * j) * D:(2 * j + 1) * D]
        bq_arr[64:128, j] = btq[(2 * j + 1) * D:(2 * j + 2) * D]
        bqs_arr[0:64, j] = bsq[(2 * j) * D:(2 * j + 1) * D]
        bqs_arr[64:128, j] = bsq[(2 * j + 1) * D:(2 * j + 2) * D]
    for h in range(HPC):
        hs = slice(h * D, (h + 1) * D)
        bk_arr[0:64, h] = btk[hs] + lts * bsk[hs]
        bk_arr[64:128, h] = lst * btk[hs] + lss * bsk[hs]

    return {
        "xtT": np.ascontiguousarray(xt[b].T),
        "xsT": np.ascontiguousarray(xs[b].T),
        "wq": chunked(wq_full, EC),
        "wqs": chunked(wqs_full, EC),
        "wk": chunked(wk_full, 2 * EC),
        "wv": chunked(wv_full, EC),
        "wc": chunked(Wc[c0:c0 + HPC * D, :], 2, ml_dtypes.bfloat16),
        "bq": bq_arr,
        "bqs": bqs_arr,
        "bk": bk_arr,
        "ones": np.ones((128, NCHUNK, 2, 64), ml_dtypes.bfloat16),
    }


def kernel(**inputs):
    xt = np.asarray(inputs["xt"], np.float32)
    xs = np.asarray(inputs["xs"], np.float32)
    Wc = np.asarray(inputs["Wc"], np.float32)
    bt = np.asarray(inputs["bt"], np.float32)
    bc = np.asarray(inputs["bc"], np.float32)
    args = dict(
        xt=xt, xs=xs,
        Wt=np.asarray(inputs["Wt"], np.float32),
        bt=bt,
        Ws=np.asarray(inputs["Ws"], np.float32),
        bs=np.asarray(inputs["bs"], np.float32),
        Wc=Wc, bc=bc,
        lam_ts=np.asarray(inputs["lam_ts"], np.float32),
        lam_st=np.asarray(inputs["lam_st"], np.float32),
        lam_ss=np.asarray(inputs["lam_ss"], np.float32),
    )
    in_maps = [_prep_core_inputs(c, **args) for c in range(NCORES)]
    nc = _get_nc()
    res = run_bass_kernel_spmd(nc, in_maps, list(range(NCORES)))
    out = np.zeros((B, L, E), np.float32)
    for c in range(NCORES):
        out[c // HPC] += res.results[c]["out"]
    # v-bias and c-bias folded in on the host: softmax rows sum to one, so
    # the v bias contributes bv @ Wc (a constant row) to every position.
    out += bt[2 * E:] @ Wc + bc
    return out


# revision 9
# speedup vs baseline: 1.2050x; 1.2050x over previous
"""Disentangled spatial attention TRN2 kernel (8 NeuronCores).

Sharding: 8 cores = 2 batches x 4 head-groups (4 heads each).
Per core, transposed-activation layout:
  qcat[h] (128, L):  rows 0:64 qt_h, rows 64:128 qs_h
  kcat[h] (128, L):  rows 0:64 k1_h = kt + lam_ts*ks,
                     rows 64:128 k2_h = lam_st*kt + lam_ss*ks
  scores^T chunk = kcat_chunk.T @ qcat  (both reference score einsums
  fused into one K=128 matmul; lam_* folded into weight shards on host)
  softmax row-sums ride along the PV matmul as 64 replicated "ones"
  columns of the v operand; normalization happens on the way into the
  transposed y layout that feeds the output projection.
Phase-1 matmuls run in float32r; attention + projection operands are
bf16 (fp32 PSUM accumulation).  v/c biases are folded in on the host
(exact: softmax rows sum to 1), qkv biases are added on device.
"""
import numpy as np
import ml_dtypes
import concourse.bass as bass
import concourse.mybir as mybir
import concourse.tile as tile
from concourse.bass_utils import run_bass_kernel_spmd

F32 = mybir.dt.float32
F32R = mybir.dt.float32r
BF16 = mybir.dt.bfloat16
AF = mybir.ActivationFunctionType

B, L, E, H, D = 2, 2048, 1024, 16, 64
HPC = 4          # heads per core
NCORES = 8
LTB = 512        # L block for phase 1
NLTB = L // LTB  # 4
NCHUNK = L // 128  # 16 Lk chunks
EC = E // 128    # 8 E chunks


def _split_multi_waits(nc, max_waits=1):
    """walrus codegen allows only one sync wait per instruction; move extra
    waits onto standalone same-engine NoOps placed just before."""
    n_split = 0
    for f in nc.m.functions:
        for blk in f.blocks:
            insts = list(blk.instructions)
            out = []
            changed = False
            for inst in insts:
                si = inst.sync_info
                waits = list(si.on_wait) if si is not None and si.on_wait else []
                if len(waits) > max_waits:
                    keep = waits[-max_waits:]
                    extra = waits[:-max_waits]
                    for w in extra:
                        nop = mybir.InstNoOp(
                            name=f"{inst.name}-wsplit{n_split}",
                            engine=inst.engine,
                            ins=[], outs=[],
                            sync_info=mybir.SyncInfo(on_wait=[w], on_update=[]),
                        )
                        out.append(nop)
                        n_split += 1
                    inst.sync_info = mybir.SyncInfo(
                        on_wait=keep,
                        on_update=list(si.on_update) if si.on_update else [],
                    )
                    changed = True
                out.append(inst)
            if changed:
                blk.instructions = out
    return n_split


def _build():
    nc = bass.Bass()
    xtT = nc.declare_dram_parameter("xtT", [E, L], BF16, isOutput=False)
    xsT = nc.declare_dram_parameter("xsT", [E, L], BF16, isOutput=False)
    wq = nc.declare_dram_parameter("wq", [128, EC, HPC * D], BF16, isOutput=False)
    wqs = nc.declare_dram_parameter("wqs", [128, EC, HPC * D], BF16, isOutput=False)
    wk = nc.declare_dram_parameter("wk", [128, 2 * EC, 2 * HPC * D], BF16, isOutput=False)
    wv = nc.declare_dram_parameter("wv", [128, EC, HPC * D], BF16, isOutput=False)
    wc = nc.declare_dram_parameter("wc", [128, 2, E], BF16, isOutput=False)
    bq = nc.declare_dram_parameter("bq", [128, 2], F32, isOutput=False)
    bqs = nc.declare_dram_parameter("bqs", [128, 2], F32, isOutput=False)
    bk = nc.declare_dram_parameter("bk", [128, HPC], F32, isOutput=False)
    ones = nc.declare_dram_parameter("ones", [128, NCHUNK, 2, 64], BF16,
                                     isOutput=False)
    out = nc.declare_dram_parameter("out", [L, E], F32, isOutput=True)

    xtT_v = xtT.rearrange("(k p) l -> p k l", p=128)   # (128, 8, L)
    xsT_v = xsT.rearrange("(k p) l -> p k l", p=128)

    with tile.TileContext(nc) as tc:
        with tc.tile_pool(name="wpool", bufs=1) as wpool, \
             tc.tile_pool(name="persist", bufs=1) as pp:
            qcat = [pp.tile([128, L], BF16, tag=f"qcat{h}", name=f"qcat{h}")
                    for h in range(HPC)]
            kcat = [pp.tile([128, L], BF16, tag=f"kcat{h}", name=f"kcat{h}")
                    for h in range(HPC)]
            # v_aug: (128, chunk, head, 128); head slot s=0: [ones | v],
            # s=1: [v | ones]
            v_sb = pp.tile([128, NCHUNK, HPC, 128], BF16, name="v_sb")
            yT = [pp.tile([128, L], BF16, tag=f"yT{j}", name=f"yT{j}")
                  for j in range(2)]
            qstg = [pp.tile([128, L], BF16, tag=f"qstg{j}", name=f"qstg{j}")
                    for j in range(2)]
            xt_sb = pp.tile([128, EC, L], BF16, name="xt_sb")
            xs_sb = pp.tile([128, EC, L], BF16, name="xs_sb")

            wq_sb = wpool.tile([128, EC, HPC * D], BF16)
            wqs_sb = wpool.tile([128, EC, HPC * D], BF16)
            wk_sb = wpool.tile([128, 2 * EC, 2 * HPC * D], BF16)
            wv_sb = wpool.tile([128, EC, HPC * D], BF16)
            bq_sb = wpool.tile([128, 2], F32)
            bqs_sb = wpool.tile([128, 2], F32)
            bk_sb = wpool.tile([128, HPC], F32)
            wc_sb = wpool.tile([128, 2, E], BF16)

            nc.sync.dma_start(xt_sb[:], xtT_v[:])
            nc.sync.dma_start(wv_sb[:], wv[:])
            nc.sync.dma_start(wq_sb[:], wq[:])
            nc.sync.dma_start(bq_sb[:], bq[:])
            nc.sync.dma_start(v_sb[:, :, 0::2, 0:64], ones[:])
            nc.sync.dma_start(v_sb[:, :, 1::2, 64:128], ones[:])
            nc.sync.dma_start(xs_sb[:], xsT_v[:])
            nc.sync.dma_start(wqs_sb[:], wqs[:])
            nc.sync.dma_start(bqs_sb[:], bqs[:])
            nc.sync.dma_start(wk_sb[:], wk[:])
            nc.sync.dma_start(bk_sb[:], bk[:])
            nc.sync.dma_start(wc_sb[:], wc[:])

            # ---- v first (only needs xt; overlaps the xs/wk loads) ----
            with tc.tile_pool(name="pvp", bufs=2, space="PSUM") as pvp:
                for ck in range(NCHUNK):
                    pv = pvp.tile([128, HPC * D], F32, tag="pv", name=f"pv{ck}")
                    for k in range(EC):
                        nc.tensor.matmul(
                            pv[:], xt_sb[:, k, ck * 128:(ck + 1) * 128],
                            wv_sb[:, k, :],
                            start=(k == 0), stop=(k == EC - 1),
                            skip_group_check=True)
                    pv_v = pv.rearrange("p (h d) -> p h d", d=D)
                    nc.vector.tensor_copy(v_sb[:, ck, 0::2, 64:128],
                                          pv_v[:, 0::2, :])
                    nc.vector.tensor_copy(v_sb[:, ck, 1::2, 0:64],
                                          pv_v[:, 1::2, :])

            # ---- head pairs: QKV then attention, interleaved ----
            with tc.tile_pool(name="p1ps", bufs=2, space="PSUM") as p1p, \
                 tc.tile_pool(name="expp", bufs=6) as expp, \
                 tc.tile_pool(name="np2", bufs=2) as np2, \
                 tc.tile_pool(name="p2s", bufs=2, space="PSUM") as p2s, \
                 tc.tile_pool(name="p2y", bufs=2, space="PSUM") as p2y:
                for j in range(2):
                    # qt / qs for the pair
                    for lt in range(4):
                        ls = slice(lt * 512, (lt + 1) * 512)
                        pq = p1p.tile([128, 512], F32, tag="p1",
                                      name=f"pq{j}{lt}")
                        for k in range(EC):
                            nc.tensor.matmul(
                                pq[:], wq_sb[:, k, j * 128:(j + 1) * 128],
                                xt_sb[:, k, ls],
                                start=(k == 0), stop=(k == EC - 1),
                                skip_group_check=True)
                        nc.vector.tensor_scalar_add(
                            qcat[2 * j][0:64, ls], pq[0:64, :],
                            bq_sb[0:64, j:j + 1])
                        nc.vector.tensor_scalar_add(
                            qstg[j][64:128, ls], pq[64:128, :],
                            bq_sb[64:128, j:j + 1])
                    for lt in range(4):
                        ls = slice(lt * 512, (lt + 1) * 512)
                        pq = p1p.tile([128, 512], F32, tag="p1",
                                      name=f"pqs{j}{lt}")
                        for k in range(EC):
                            nc.tensor.matmul(
                                pq[:], wqs_sb[:, k, j * 128:(j + 1) * 128],
                                xs_sb[:, k, ls],
                                start=(k == 0), stop=(k == EC - 1),
                                skip_group_check=True)
                        nc.vector.tensor_scalar_add(
                            qstg[j][0:64, ls], pq[0:64, :],
                            bqs_sb[0:64, j:j + 1])
                        nc.vector.tensor_scalar_add(
                            qcat[2 * j + 1][64:128, ls], pq[64:128, :],
                            bqs_sb[64:128, j:j + 1])
                    nc.sync.dma_start(qcat[2 * j + 1][0:64, :],
                                      qstg[j][64:128, :])
                    nc.sync.dma_start(qcat[2 * j][64:128, :], qstg[j][0:64, :])

                    for h in (2 * j, 2 * j + 1):
                        # kcat[h]
                        for lt in range(4):
                            ls = slice(lt * 512, (lt + 1) * 512)
                            pkh = p1p.tile([128, 512], F32, tag="p1",
                                           name=f"pk{h}{lt}")
                            for k in range(EC):
                                nc.tensor.matmul(
                                    pkh[:], wk_sb[:, k, h * 128:(h + 1) * 128],
                                    xt_sb[:, k, ls],
                                    start=(k == 0), stop=False,
                                    skip_group_check=True)
                            for k in range(EC):
                                nc.tensor.matmul(
                                    pkh[:], wk_sb[:, EC + k, h * 128:(h + 1) * 128],
                                    xs_sb[:, k, ls],
                                    start=False, stop=(k == EC - 1),
                                    skip_group_check=True)
                            nc.vector.tensor_scalar_add(
                                kcat[h][:, ls], pkh[:], bk_sb[:, h:h + 1])

                        # attention for head h
                        s = h % 2
                        sums_h = slice(0, 64) if s == 0 else slice(64, 128)
                        y_h = slice(64, 128) if s == 0 else slice(0, 64)
                        slot = slice(0, 64) if s == 0 else slice(64, 128)
                        for lq in range(4):
                            qs_ = slice(lq * 512, (lq + 1) * 512)
                            py = p2y.tile([128, 512], F32, tag="py",
                                          name=f"py{h}{lq}")
                            for g in range(8):
                                ps = p2s.tile([128, 1024], F32, tag="ps",
                                              name=f"ps{h}{lq}{g}")
                                for hf in range(2):
                                    ck = 2 * g + hf
                                    nc.tensor.matmul(
                                        ps[:, hf * 512:(hf + 1) * 512],
                                        kcat[h][:, ck * 128:(ck + 1) * 128],
                                        qcat[h][:, qs_],
                                        start=True, stop=True,
                                        skip_group_check=True)
                                ex = expp.tile([128, 1024], BF16, tag="ex",
                                               name=f"ex{h}{lq}{g}")
                                nc.scalar.activation(ex[:], ps[:], AF.Exp,
                                                     scale=0.125)
                                for hf in range(2):
                                    ck = 2 * g + hf
                                    nc.tensor.matmul(
                                        py[:], v_sb[:, ck, h, :],
                                        ex[:, hf * 512:(hf + 1) * 512],
                                        start=(ck == 0),
                                        stop=(ck == NCHUNK - 1),
                                        skip_group_check=True)
                            lnt = np2.tile([128, 512], F32, tag="lnt",
                                           name=f"ln{h}{lq}")
                            nc.scalar.activation(lnt[sums_h, :],
                                                 py[sums_h, :], AF.Ln)
                            rec = np2.tile([128, 512], F32, tag="rec",
                                           name=f"rec{h}{lq}")
                            nc.scalar.activation(rec[sums_h, :],
                                                 lnt[sums_h, :], AF.Exp,
                                                 scale=-1.0)
                            rec2 = np2.tile([128, 512], F32, tag="rec2",
                                            name=f"rec2{h}{lq}")
                            nc.sync.dma_start(rec2[y_h, :], rec[sums_h, :])
                            yst = np2.tile([128, 512], BF16, tag="yst",
                                           name=f"yst{h}{lq}")
                            nc.vector.tensor_tensor(yst[y_h, :], py[y_h, :],
                                                    rec2[y_h, :],
                                                    mybir.AluOpType.mult)
                            nc.sync.dma_start(yT[j][slot, qs_], yst[y_h, :])

            # ---------------- projection ----------------
            with tc.tile_pool(name="outp", bufs=3) as outp, \
                 tc.tile_pool(name="p3o", bufs=4, space="PSUM") as p3o:
                for lqt in range(L // 128):
                    lqs = slice(lqt * 128, (lqt + 1) * 128)
                    ot = outp.tile([128, E], F32, tag="ot", name=f"ot{lqt}")
                    for nch in range(2):
                        ns = slice(nch * 512, (nch + 1) * 512)
                        po = p3o.tile([128, 512], F32, tag="po",
                                      name=f"po{lqt}{nch}")
                        nc.tensor.matmul(po[:], yT[0][:, lqs], wc_sb[:, 0, ns],
                                         start=True, stop=False,
                                         skip_group_check=True)
                        nc.tensor.matmul(po[:], yT[1][:, lqs], wc_sb[:, 1, ns],
                                         start=False, stop=True,
                                         skip_group_check=True)
                        if nch == 0:
                            nc.vector.tensor_copy(ot[:, ns], po[:])
                        else:
                            nc.scalar.copy(ot[:, ns], po[:])
                    nc.sync.dma_start(out[lqs, :], ot[:])
    return nc


_NC_CACHE = None


def _get_nc():
    global _NC_CACHE
    if _NC_CACHE is None:
        nc = _build()
        _split_multi_waits(nc)
        _NC_CACHE = nc
    return _NC_CACHE


def _prep_core_inputs(core, xt, xs, Wt, bt, Ws, bs, Wc, bc, lam_ts, lam_st,
                      lam_ss):
    b, hg = core // HPC, core % HPC
    c0 = hg * HPC * D  # 256*hg
    lts, lst, lss = float(lam_ts[0]), float(lam_st[0]), float(lam_ss[0])

    wq_full = Wt[:, c0:c0 + HPC * D]                     # (E, 256) qt
    wqs_full = Ws[:, c0:c0 + HPC * D]                    # (E, 256) qs
    wv_full = Wt[:, 2 * E + c0:2 * E + c0 + HPC * D]     # (E, 256)
    ktw = Wt[:, E + c0:E + c0 + HPC * D]                 # (E, 256)
    ksw = Ws[:, E + c0:E + c0 + HPC * D]                 # (E, 256)

    wk_full = np.zeros((2 * E, 2 * HPC * D), np.float32)
    for h in range(HPC):
        hs = slice(h * D, (h + 1) * D)
        wk_full[:E, h * 128:h * 128 + D] = ktw[:, hs]
        wk_full[:E, h * 128 + D:(h + 1) * 128] = lst * ktw[:, hs]
        wk_full[E:, h * 128:h * 128 + D] = lts * ksw[:, hs]
        wk_full[E:, h * 128 + D:(h + 1) * 128] = lss * ksw[:, hs]

    def chunked(a, nk, dtype=np.float32):
        return np.ascontiguousarray(
            a.reshape(nk, 128, a.shape[1]).transpose(1, 0, 2)).astype(dtype)

    btq = bt[c0:c0 + HPC * D]
    bsq = bs[c0:c0 + HPC * D]
    btk = bt[E + c0:E + c0 + HPC * D]
    bsk = bs[E + c0:E + c0 + HPC * D]
    bq_arr = np.zeros((128, 2), np.float32)
    bqs_arr = np.zeros((128, 2), np.float32)
    bk_arr = np.zeros((128, HPC), np.float32)
    for j in range(2):
        bq_arr[0:64, j] = btq[(2 * j) * D:(2 * j + 1) * D]
        bq_arr[64:128, j] = btq[(2 * j + 1) * D:(2 * j + 2) * D]
        bqs_arr[0:64, j] = bsq[(2 * j) * D:(2 * j + 1) * D]
        bqs_arr[64:128, j] = bsq[(2 * j + 1) * D:(2 * j + 2) * D]
    for h in range(HPC):
        hs = slice(h * D, (h + 1) * D)
        bk_arr[0:64, h] = btk[hs] + lts * bsk[hs]
        bk_arr[64:128, h] = lst * btk[hs] + lss * bsk[hs]

    return {
        "xtT": np.ascontiguousarray(xt[b].T).astype(ml_dtypes.bfloat16),
        "xsT": np.ascontiguousarray(xs[b].T).astype(ml_dtypes.bfloat16),
        "wq": chunked(wq_full, EC, ml_dtypes.bfloat16),
        "wqs": chunked(wqs_full, EC, ml_dtypes.bfloat16),
        "wk": chunked(wk_full, 2 * EC, ml_dtypes.bfloat16),
        "wv": chunked(wv_full, EC, ml_dtypes.bfloat16),
        "wc": chunked(Wc[c0:c0 + HPC * D, :], 2, ml_dtypes.bfloat16),
        "bq": bq_arr,
        "bqs": bqs_arr,
        "bk": bk_arr,
        "ones": np.ones((128, NCHUNK, 2, 64), ml_dtypes.bfloat16),
    }


def kernel(**inputs):
    xt = np.asarray(inputs["xt"], np.float32)
    xs = np.asarray(inputs["xs"], np.float32)
    Wc = np.asarray(inputs["Wc"], np.float32)
    bt = np.asarray(inputs["bt"], np.float32)
    bc = np.asarray(inputs["bc"], np.float32)
    args = dict(
        xt=xt, xs=xs,
        Wt=np.asarray(inputs["Wt"], np.float32),
        bt=bt,
        Ws=np.asarray(inputs["Ws"], np.float32),
        bs=np.asarray(inputs["bs"], np.float32),
        Wc=Wc, bc=bc,
        lam_ts=np.asarray(inputs["lam_ts"], np.float32),
        lam_st=np.asarray(inputs["lam_st"], np.float32),
        lam_ss=np.asarray(inputs["lam_ss"], np.float32),
    )
    in_maps = [_prep_core_inputs(c, **args) for c in range(NCORES)]
    nc = _get_nc()
    res = run_bass_kernel_spmd(nc, in_maps, list(range(NCORES)))
    out = np.zeros((B, L, E), np.float32)
    for c in range(NCORES):
        out[c // HPC] += res.results[c]["out"]
    # v-bias and c-bias folded in on the host: softmax rows sum to one, so
    # the v bias contributes bv @ Wc (a constant row) to every position.
    out += bt[2 * E:] @ Wc + bc
    return out
